# revision 1
# baseline (speedup 1.0000x reference)
"""Trainium2 Bass kernel for nn_Decoder (512-step LSTM scan, B=256, F=256).

Strategy: data-parallel over batch across 8 NeuronCores (32 batch/core).
After step 0 the LSTM input equals the hidden state, so W_ih+W_hh fold into
one combined weight for steps 1..511. Step 0 (and the initial_layer) runs on
host in numpy; each core runs 511 uniform recurrence steps.

Per-step device dataflow (batch-major [32, *] tiles):
  PE   : gates = hT.T @ wcT (+bias via ones-row matmul), fp32r, 6 MMs
  ACT  : sigma(i,f), sigma(o), tanh(g), tanh(c2)
  DVE  : c2 = sig_f*c + sig_i*tanh_g ; h2 = sig_o*tanh(c2)
  PE   : transpose h2 -> hT for the next step's stationary operand
"""
import sys

sys.path.insert(0, "/opt/trn_rl_repo")

import numpy as np

SEQ_LEN = 512
B, L, F = 256, 128, 256
NCORES = 8
BS = B // NCORES  # 32 batch per core

_CACHE = {}
VERSION = 3  # bump on every program change: forces a distinct NEFF cache key


def _sigmoid(x):
    out = np.empty_like(x)
    pos = x >= 0
    out[pos] = 1.0 / (1.0 + np.exp(-x[pos]))
    e = np.exp(x[~pos])
    out[~pos] = e / (1.0 + e)
    return out


def _build(steps):
    """Build + schedule the per-core Bass program (same program all cores)."""
    import concourse.mybir as mybir
    import concourse.tile as tile
    from concourse import bacc
    from concourse.masks import make_identity

    f32 = mybir.dt.float32
    f32r = mybir.dt.float32r
    AF = mybir.ActivationFunctionType

    nc = bacc.Bacc("TRN2", target_bir_lowering=False, debug=False)

    hT0_d = nc.dram_tensor("hT0", [F, BS], f32, kind="ExternalInput")
    c1_d = nc.dram_tensor("c1", [BS, F], f32, kind="ExternalInput")
    wcT_d = nc.dram_tensor("wcT", [F, 4 * F], f32, kind="ExternalInput")
    bias_d = nc.dram_tensor("bias", [1, 4 * F], f32, kind="ExternalInput")
    ones_d = nc.dram_tensor("ones", [1, BS], f32, kind="ExternalInput")
    # cache-buster: the neuron NEFF cache key ignores backend_config (the BIR),
    # so distinct programs with identical I/O shapes collide. Unique shape per
    # (VERSION, steps) forces a distinct HLO and cache entry.
    stag_d = nc.dram_tensor("stag", [VERSION, steps], f32, kind="ExternalInput")
    outs_d = nc.dram_tensor("outs", [SEQ_LEN, BS, F], f32, kind="ExternalOutput")

    with tile.TileContext(nc) as tc:
        with tc.tile_pool(name="const", bufs=1) as cpool, \
             tc.tile_pool(name="state", bufs=2) as spool, \
             tc.tile_pool(name="work", bufs=2) as wpool, \
             tc.tile_pool(name="h2p", bufs=6) as hpool, \
             tc.tile_pool(name="ps", bufs=2, space="PSUM") as psp:

            wc_sb = cpool.tile([128, 2 * 4 * F], f32r)
            nc.gpsimd.dma_start(
                out=wc_sb[:].rearrange("p (k n) -> p k n", k=2),
                in_=wcT_d.ap().rearrange("(k p) n -> p k n", p=128))
            bias_sb = cpool.tile([1, 4 * F], f32r)
            nc.gpsimd.dma_start(out=bias_sb[:], in_=bias_d.ap())
            ones_sb = cpool.tile([1, BS], f32r)
            nc.gpsimd.dma_start(out=ones_sb[:], in_=ones_d.ap())
            ident = cpool.tile([32, 32], f32)
            make_identity(nc, ident)
            stag_sb = cpool.tile([1, 1], f32)
            nc.sync.dma_start(out=stag_sb[:], in_=stag_d.ap()[0:1, 0:1])

            hT_cur = spool.tile([128, 2 * BS], f32r, tag="hT")
            nc.gpsimd.dma_start(
                out=hT_cur[:].rearrange("p (k b) -> p k b", k=2),
                in_=hT0_d.ap().rearrange("(k p) b -> p k b", p=128))
            c_cur = spool.tile([BS, F], f32, tag="c")
            nc.sync.dma_start(out=c_cur[:], in_=c1_d.ap())

            for t in range(1, steps + 1):
                psA = psp.tile([BS, 512], f32, tag="psA")
                psB = psp.tile([BS, 512], f32, tag="psB")
                for ps, off in ((psA, 0), (psB, 512)):
                    for k in range(2):
                        nc.tensor.matmul(
                            ps[:], lhsT=hT_cur[:, BS * k:BS * (k + 1)],
                            rhs=wc_sb[:, 4 * F * k + off: 4 * F * k + off + 512],
                            start=(k == 0), stop=False)
                    nc.tensor.matmul(
                        ps[:], lhsT=ones_sb[:], rhs=bias_sb[:, off:off + 512],
                        start=False, stop=True)

                sA = wpool.tile([BS, 512], f32, tag="sA")
                nc.scalar.activation(sA[:], psA[:], AF.Sigmoid)
                sO = wpool.tile([BS, F], f32, tag="sO")
                nc.scalar.activation(sO[:], psB[:, 0:F], AF.Sigmoid)
                tg = wpool.tile([BS, F], f32, tag="tg")
                nc.scalar.activation(tg[:], psB[:, F:2 * F], AF.Tanh)

                t2 = wpool.tile([BS, F], f32, tag="t2")
                nc.vector.tensor_mul(t2[:], sA[:, F:2 * F], c_cur[:])
                t1 = wpool.tile([BS, F], f32, tag="t1")
                nc.vector.tensor_mul(t1[:], sA[:, 0:F], tg[:])
                c_new = spool.tile([BS, F], f32, tag="c")
                nc.vector.tensor_add(c_new[:], t1[:], t2[:])
                tc_t = wpool.tile([BS, F], f32, tag="tc")
                nc.scalar.activation(tc_t[:], c_new[:], AF.Tanh)
                h2 = hpool.tile([BS, F], f32, tag="h2")
                nc.vector.tensor_mul(h2[:], sO[:], tc_t[:])

                nc.sync.dma_start(out=outs_d.ap()[t], in_=h2[:])

                if t < steps:
                    tps = psp.tile([128, 2 * BS], f32, tag="tps")
                    for k in range(2):
                        nc.tensor.transpose(
                            tps[:, BS * k:BS * (k + 1)],
                            h2[:, 128 * k:128 * (k + 1)], ident[:])
                    hT_new = spool.tile([128, 2 * BS], f32r, tag="hT")
                    nc.scalar.copy(hT_new[:], tps[:])
                    hT_cur = hT_new
                c_cur = c_new

    nc.compile()
    return nc


def _get_nc(steps):
    if steps not in _CACHE:
        _CACHE[steps] = _build(steps)
    return _CACHE[steps]


def _host_prep(x, last_feat, Wi, bi, W_ih, W_hh, b_ih, b_hh):
    x = np.asarray(x, np.float32)
    last_feat = np.asarray(last_feat, np.float32)
    Wi = np.asarray(Wi, np.float32); bi = np.asarray(bi, np.float32)
    W_ih = np.asarray(W_ih, np.float32); W_hh = np.asarray(W_hh, np.float32)
    b_ih = np.asarray(b_ih, np.float32); b_hh = np.asarray(b_hh, np.float32)

    z = x[0] @ Wi.T + bi                       # [B, F]
    init = np.where(z > 0, z, np.expm1(z)).astype(np.float32)  # elu

    bsum = b_ih + b_hh
    g0 = last_feat @ W_ih.T + init @ W_hh.T + bsum   # [B, 4F] order i,f,g,o
    i0, f0, g0g, o0 = (g0[:, 0:F], g0[:, F:2*F], g0[:, 2*F:3*F], g0[:, 3*F:4*F])
    c1 = _sigmoid(f0) * init + _sigmoid(i0) * np.tanh(g0g)
    h1 = (_sigmoid(o0) * np.tanh(c1)).astype(np.float32)
    c1 = c1.astype(np.float32)

    # combined recurrent weight, rows reordered [i, f, o, g]
    Wc = W_ih + W_hh                            # [4F, F]
    perm = np.concatenate([np.arange(0, F), np.arange(F, 2*F),
                           np.arange(3*F, 4*F), np.arange(2*F, 3*F)])
    wcT = np.ascontiguousarray(Wc[perm].T)      # [F, 4F] gate order i,f,o,g
    bias_row = np.ascontiguousarray(bsum[perm][None, :])  # [1, 4F]
    return h1, c1, wcT, bias_row


_steps_of = [SEQ_LEN - 1]


def _in_maps(inputs, steps=None):
    _steps_of[0] = steps or _steps_of[0]
    h1, c1, wcT, bias_row = _host_prep(
        inputs["x"], inputs["last_feat"], inputs["Wi"], inputs["bi"],
        inputs["W_ih"], inputs["W_hh"], inputs["b_ih"], inputs["b_hh"])
    ones = np.ones((1, BS), np.float32)
    maps = []
    for ci in range(NCORES):
        s = slice(ci * BS, (ci + 1) * BS)
        maps.append(dict(
            hT0=np.ascontiguousarray(h1[s].T),
            c1=np.ascontiguousarray(c1[s]),
            wcT=wcT, bias=bias_row, ones=ones,
            stag=np.zeros((VERSION, _steps_of[0]), np.float32)))
    return maps


def kernel(x, last_feat, Wi, bi, W_ih, W_hh, b_ih, b_hh, Wo, bo,
           _steps=SEQ_LEN - 1):
    from concourse.bass_utils import run_bass_kernel_spmd

    h1, c1, wcT, bias_row = _host_prep(x, last_feat, Wi, bi, W_ih, W_hh,
                                       b_ih, b_hh)
    ones = np.ones((1, BS), np.float32)
    in_maps = []
    for ci in range(NCORES):
        s = slice(ci * BS, (ci + 1) * BS)
        in_maps.append(dict(
            hT0=np.ascontiguousarray(h1[s].T),
            c1=np.ascontiguousarray(c1[s]),
            wcT=wcT, bias=bias_row, ones=ones))

    for m in in_maps:
        m["stag"] = np.zeros((VERSION, _steps), np.float32)
    nc = _get_nc(_steps)
    res = run_bass_kernel_spmd(nc, in_maps, core_ids=list(range(NCORES)))

    outs = np.concatenate([r["outs"] for r in res.results], axis=1)  # [S, B, F]
    outs[0] = h1
    return np.ascontiguousarray(outs).reshape(B, SEQ_LEN, F)



# revision 32
# speedup vs baseline: 188.4264x; 188.4264x over previous
"""Trainium2 Bass kernel for nn_Decoder (512-step LSTM scan, B=256, F=256).

Strategy: data-parallel over batch across 8 NeuronCores (32 batch/core).
After step 0 the LSTM input equals the hidden state, so W_ih+W_hh fold into
one combined weight for steps 1..511. Step 0 (and the initial_layer) runs on
host in numpy; each core runs 511 uniform recurrence steps.

v2: feature-major (transposed) layout + hardware For_i loop.
  - State lives transposed: hT/cT are [128 partitions, 2*32] (feature-chunk
    k major, batch minor), so every elementwise op uses all 128 lanes.
  - gatesT [1024, 32] accumulates in one PSUM tile [128, 8*32]: 8 gate-row
    chunks x (2 K-chunk matmuls with the weight stationary) + one K=8
    masked rank-8 matmul that adds the bias exactly (bias * {0,1}).
  - Weight-stationary matmuls stream only 32 columns each; products and
    accumulation order match the baseline exactly (k0 + k1, then bias), so
    the fp32 trajectory is unchanged.
  - The whole scan runs inside tc.For_i unrolled 7 steps per iteration:
    program size is O(1) in steps, killing the per-call NEFF-size overhead
    that dominated the unrolled baseline.
  - Output h_t is transposed back to batch-major on the PE (idle between
    gate bursts), copied PSUM->SBUF on Pool, and DMA'd per step from SP.
"""
import sys

sys.path.insert(0, "/opt/trn_rl_repo")

import numpy as np

SEQ_LEN = 512
B, L, F = 256, 128, 256
NCORES = 8
BS = B // NCORES  # 32 batch per core
UNROLL = 7        # 511 = 73 * 7 -> no tail for the full problem

_CACHE = {}
VERSION = 32  # bump on every program change: forces a distinct NEFF cache key


def _sigmoid(x):
    out = np.empty_like(x)
    pos = x >= 0
    out[pos] = 1.0 / (1.0 + np.exp(-x[pos]))
    e = np.exp(x[~pos])
    out[~pos] = e / (1.0 + e)
    return out


def _build(steps, reps=1, timing=False):
    """Build + schedule the per-core Bass program (same program all cores).

    reps>1 re-runs the scan body reps*steps times inside the same hardware
    loop (for timing amplification); output rows past SEQ_LEN are skipped
    by DMA bounds checks, and rows 0..SEQ_LEN-1 are still written exactly
    once with correct values.
    """
    import concourse.mybir as mybir
    import concourse.tile as tile
    from concourse import bacc
    from concourse.bass import ds
    from concourse.expressions import smin

    f32 = mybir.dt.float32
    f32r = mybir.dt.float32r
    AF = mybir.ActivationFunctionType

    nc = bacc.Bacc("TRN2", target_bir_lowering=False, debug=False)

    hT0_d = nc.dram_tensor("hT0", [128, 2 * BS], f32, kind="ExternalInput")
    cT0_d = nc.dram_tensor("cT0", [128, 2 * BS], f32, kind="ExternalInput")
    wc_d = nc.dram_tensor("wc", [128, 2048], f32, kind="ExternalInput")
    b8_d = nc.dram_tensor("b8", [8, 128], f32, kind="ExternalInput")
    m8_d = nc.dram_tensor("m8", [8, 256], f32, kind="ExternalInput")
    # cache-buster: the neuron NEFF cache key ignores backend_config (the BIR),
    # so distinct programs with identical I/O shapes collide. Unique shape per
    # (VERSION, steps) forces a distinct HLO and cache entry.
    stag_d = nc.dram_tensor("stag", [VERSION, steps + reps], f32, kind="ExternalInput")
    # output stored feature-major per step: rows t*F + (k*128+p), cols b;
    # host transposes back to [S, BS, F] afterwards (steps-independent cost)
    out_rows = F if timing else SEQ_LEN * F
    outs_d = nc.dram_tensor("outs", [out_rows, BS], f32, kind="ExternalOutput")

    with tile.TileContext(nc) as tc:
        with tc.tile_pool(name="const", bufs=1) as cpool, \
             tc.tile_pool(name="state", bufs=1) as spool, \
             tc.tile_pool(name="work", bufs=2) as wpool, \
             tc.tile_pool(name="obuf", bufs=4) as opool, \
             tc.tile_pool(name="ps", bufs=2, space="PSUM") as psp:

            wc_sb = cpool.tile([128, 2048], f32r)
            nc.gpsimd.dma_start(out=wc_sb[:], in_=wc_d.ap())
            b8_sb = cpool.tile([8, 128], f32r)
            nc.gpsimd.dma_start(out=b8_sb[:], in_=b8_d.ap())
            m8_sb = cpool.tile([8, 256], f32r)
            nc.gpsimd.dma_start(out=m8_sb[:], in_=m8_d.ap())
            stag_sb = cpool.tile([1, 1], f32)
            nc.sync.dma_start(out=stag_sb[:], in_=stag_d.ap()[0:1, 0:1])

            h_state = spool.tile([128, 2 * BS], f32r, tag="h")
            nc.gpsimd.dma_start(out=h_state[:], in_=hT0_d.ap())
            c_state = spool.tile([128, 2 * BS], f32, tag="c")
            nc.sync.dma_start(out=c_state[:], in_=cT0_d.ap())

            def emit_out(row_off):
                """Stage pre-update h_state (Pool) and DMA it feature-major.

                row_off is in units of outs_d rows (t * F).
                """
                stg = opool.tile([128, 2 * BS], f32, tag="stg")
                nc.gpsimd.tensor_copy(stg[:], h_state[:])
                if timing:
                    dst = outs_d.ap()[0:F]
                elif isinstance(row_off, int):
                    dst = outs_d.ap()[row_off:row_off + F]
                else:
                    dst = outs_d.ap()[ds(row_off, F)]
                nc.sync.dma_start(
                    out=dst.rearrange("(k p) b -> p k b", k=2),
                    in_=stg[:].rearrange("p (k b) -> p k b", k=2))

            def step(row_off):
                # start=True zeroes the WHOLE PSUM bank (bank-granular reset),
                # so only the first matmul of the step may carry it; all other
                # accumulations ride on the zeroed bank. k-major order lets
                # PE begin the 8 k0 matmuls as soon as h chunk0 is written.
                # Per-element accumulate order stays k0, k1, bias (= baseline).
                psT = psp.tile([128, 256], f32, tag="psT")
                for k in range(2):
                    for j in range(8):
                        nc.tensor.matmul(
                            psT[:, 32 * j:32 * (j + 1)],
                            lhsT=wc_sb[:, 1024 * k + 128 * j:1024 * k + 128 * (j + 1)],
                            rhs=h_state[:, BS * k:BS * (k + 1)],
                            start=(j == 0 and k == 0), stop=False,
                            skip_group_check=True)
                nc.tensor.matmul(psT[:], lhsT=b8_sb[:], rhs=m8_sb[:],
                                 start=False, stop=True, skip_group_check=True)
                # output of the PREVIOUS step (pre-update h_state): staged on
                # Pool + DMA'd; fully off the recurrence chain.
                in_bounds = not (isinstance(row_off, int)
                                 and row_off >= SEQ_LEN * F)
                if in_bounds:
                    emit_out(row_off)

                sg = wpool.tile([128, 192], f32, tag="sg")
                nc.scalar.activation(sg[:, 0:128], psT[:, 0:128], AF.Sigmoid)
                tg = wpool.tile([128, 64], f32, tag="tg")
                nc.scalar.activation(tg[:], psT[:, 192:256], AF.Tanh)
                nc.scalar.activation(sg[:, 128:192], psT[:, 128:192],
                                     AF.Sigmoid)
                t2 = wpool.tile([128, 64], f32, tag="t2")
                nc.gpsimd.tensor_mul(t2[:], sg[:, 64:128], c_state[:])
                t1 = wpool.tile([128, 64], f32, tag="t1")
                nc.vector.tensor_mul(t1[:], sg[:, 0:64], tg[:])
                nc.vector.tensor_add(c_state[:], t1[:], t2[:])
                tc2 = wpool.tile([128, 64], f32, tag="tc2")
                nc.scalar.activation(tc2[:], c_state[:], AF.Tanh)
                nc.vector.tensor_mul(h_state[:, 0:BS], sg[:, 128:160],
                                     tc2[:, 0:BS])
                nc.vector.tensor_mul(h_state[:, BS:2 * BS], sg[:, 160:192],
                                     tc2[:, BS:2 * BS])

            total = steps * reps
            n_loop = (total // UNROLL) * UNROLL
            if n_loop:
                with tc.For_i(0, n_loop * F, UNROLL * F) as i:
                    for u in range(UNROLL):
                        step(i + u * F)
            for u in range(total - n_loop):
                step((n_loop + u) * F)
            if total < SEQ_LEN:
                # row `steps` (= h_{steps+1}) not yet covered by a body
                emit_out(steps * F)

    nc.compile()
    return nc


def _get_nc(steps, reps=1, timing=False):
    key = (steps, reps, timing)
    if key not in _CACHE:
        _CACHE[key] = _build(steps, reps, timing)
    return _CACHE[key]


def _host_prep(x, last_feat, Wi, bi, W_ih, W_hh, b_ih, b_hh):
    x = np.asarray(x, np.float32)
    last_feat = np.asarray(last_feat, np.float32)
    Wi = np.asarray(Wi, np.float32); bi = np.asarray(bi, np.float32)
    W_ih = np.asarray(W_ih, np.float32); W_hh = np.asarray(W_hh, np.float32)
    b_ih = np.asarray(b_ih, np.float32); b_hh = np.asarray(b_hh, np.float32)

    z = x[0] @ Wi.T + bi                       # [B, F]
    init = np.where(z > 0, z, np.expm1(z)).astype(np.float32)  # elu

    bsum = b_ih + b_hh
    g0 = last_feat @ W_ih.T + init @ W_hh.T + bsum   # [B, 4F] order i,f,g,o
    i0, f0, g0g, o0 = (g0[:, 0:F], g0[:, F:2*F], g0[:, 2*F:3*F], g0[:, 3*F:4*F])
    c1 = _sigmoid(f0) * init + _sigmoid(i0) * np.tanh(g0g)
    h1 = (_sigmoid(o0) * np.tanh(c1)).astype(np.float32)
    c1 = c1.astype(np.float32)

    # combined recurrent weight, rows reordered [i, f, o, g]
    Wc = W_ih + W_hh                            # [4F, F]
    perm = np.concatenate([np.arange(0, F), np.arange(F, 2*F),
                           np.arange(3*F, 4*F), np.arange(2*F, 3*F)])
    wcT = np.ascontiguousarray(Wc[perm].T)      # [F, 4F] gate order i,f,o,g
    bias_perm = bsum[perm].astype(np.float32)   # [4F]
    return h1, c1, wcT, bias_perm


def _to_fmajor(a):
    """[BS, 256] batch-major -> [128, 2*BS] feature-chunk-major."""
    return np.ascontiguousarray(
        a.T.reshape(2, 128, BS).transpose(1, 0, 2).reshape(128, 2 * BS))


def kernel(x, last_feat, Wi, bi, W_ih, W_hh, b_ih, b_hh, Wo, bo,
           _steps=SEQ_LEN - 1, _reps=1, _timing=False):
    from concourse.bass_utils import run_bass_kernel_spmd

    h1, c1, wcT, bias_perm = _host_prep(x, last_feat, Wi, bi, W_ih, W_hh,
                                        b_ih, b_hh)
    # wc layout: wc[p, k*1024 + j*128 + q] = wcT[k*128 + p, j*128 + q]
    wc_host = np.ascontiguousarray(
        wcT.reshape(2, 128, 8, 128).transpose(1, 0, 2, 3).reshape(128, 2048))
    b8 = np.ascontiguousarray(bias_perm.reshape(8, 128))
    m8 = np.ascontiguousarray(
        np.kron(np.eye(8, dtype=np.float32), np.ones((1, BS), np.float32)))

    in_maps = []
    for ci in range(NCORES):
        s = slice(ci * BS, (ci + 1) * BS)
        in_maps.append(dict(
            hT0=_to_fmajor(h1[s]),
            cT0=_to_fmajor(c1[s]),
            wc=wc_host, b8=b8, m8=m8,
            stag=np.zeros((VERSION, _steps + _reps), np.float32)))

    nc = _get_nc(_steps, _reps, _timing)
    res = run_bass_kernel_spmd(nc, in_maps, core_ids=list(range(NCORES)))
    if _timing:
        return None

    # outs rows [t*F + f, b] = h_{t+1}[b, f]; device covers t = 0.._steps
    per_core = [r["outs"].reshape(SEQ_LEN, F, BS).transpose(0, 2, 1)
                for r in res.results]
    outs = np.concatenate(per_core, axis=1)  # [S, B, F]
    return np.ascontiguousarray(outs).reshape(B, SEQ_LEN, F)


# revision 33
# speedup vs baseline: 188.9401x; 1.0027x over previous
"""Trainium2 Bass kernel for nn_Decoder (512-step LSTM scan, B=256, F=256).

Strategy: data-parallel over batch across 8 NeuronCores (32 batch/core).
After step 0 the LSTM input equals the hidden state, so W_ih+W_hh fold into
one combined weight for steps 1..511. Step 0 (and the initial_layer) runs on
host in numpy; each core runs 511 uniform recurrence steps.

v2: feature-major (transposed) layout + hardware For_i loop.
  - State lives transposed: hT/cT are [128 partitions, 2*32] (feature-chunk
    k major, batch minor), so every elementwise op uses all 128 lanes.
  - gatesT [1024, 32] accumulates in one PSUM tile [128, 8*32]: 8 gate-row
    chunks x (2 K-chunk matmuls with the weight stationary) + one K=8
    masked rank-8 matmul that adds the bias exactly (bias * {0,1}).
  - Weight-stationary matmuls stream only 32 columns each; products and
    accumulation order match the baseline exactly (k0 + k1, then bias), so
    the fp32 trajectory is unchanged.
  - The whole scan runs inside tc.For_i unrolled 7 steps per iteration:
    program size is O(1) in steps, killing the per-call NEFF-size overhead
    that dominated the unrolled baseline.
  - Output h_t is staged to a Pool-copied SBUF tile and DMA'd per step in
    feature-major DRAM layout [S, F, BS]; the host transposes back to
    [S, B, F] once at the end (steps-independent cost).
  - Measured ~6.5 us/step on HW (8 cores in parallel, 511 steps ~ 3.3 ms)
    vs 1225 us/step for the fully unrolled baseline program.
"""
import sys

sys.path.insert(0, "/opt/trn_rl_repo")

import numpy as np

SEQ_LEN = 512
B, L, F = 256, 128, 256
NCORES = 8
BS = B // NCORES  # 32 batch per core
UNROLL = 7        # 511 = 73 * 7 -> no tail for the full problem

_CACHE = {}
VERSION = 32  # bump on every program change: forces a distinct NEFF cache key


def _sigmoid(x):
    out = np.empty_like(x)
    pos = x >= 0
    out[pos] = 1.0 / (1.0 + np.exp(-x[pos]))
    e = np.exp(x[~pos])
    out[~pos] = e / (1.0 + e)
    return out


def _build(steps, reps=1, timing=False):
    """Build + schedule the per-core Bass program (same program all cores).

    reps>1 re-runs the scan body reps*steps times inside the same hardware
    loop (for timing amplification); output rows past SEQ_LEN are skipped
    by DMA bounds checks, and rows 0..SEQ_LEN-1 are still written exactly
    once with correct values.
    """
    import concourse.mybir as mybir
    import concourse.tile as tile
    from concourse import bacc
    from concourse.bass import ds
    from concourse.expressions import smin  # noqa: F401 (used in timing reps)

    f32 = mybir.dt.float32
    f32r = mybir.dt.float32r
    AF = mybir.ActivationFunctionType

    nc = bacc.Bacc("TRN2", target_bir_lowering=False, debug=False)

    hT0_d = nc.dram_tensor("hT0", [128, 2 * BS], f32, kind="ExternalInput")
    cT0_d = nc.dram_tensor("cT0", [128, 2 * BS], f32, kind="ExternalInput")
    wc_d = nc.dram_tensor("wc", [128, 2048], f32, kind="ExternalInput")
    b8_d = nc.dram_tensor("b8", [8, 128], f32, kind="ExternalInput")
    m8_d = nc.dram_tensor("m8", [8, 256], f32, kind="ExternalInput")
    # cache-buster: the neuron NEFF cache key ignores backend_config (the BIR),
    # so distinct programs with identical I/O shapes collide. Unique shape per
    # (VERSION, steps) forces a distinct HLO and cache entry.
    stag_d = nc.dram_tensor("stag", [VERSION, steps + reps], f32, kind="ExternalInput")
    # output stored feature-major per step: rows t*F + (k*128+p), cols b;
    # host transposes back to [S, BS, F] afterwards (steps-independent cost)
    out_rows = F if timing else SEQ_LEN * F
    outs_d = nc.dram_tensor("outs", [out_rows, BS], f32, kind="ExternalOutput")

    with tile.TileContext(nc) as tc:
        with tc.tile_pool(name="const", bufs=1) as cpool, \
             tc.tile_pool(name="state", bufs=1) as spool, \
             tc.tile_pool(name="work", bufs=2) as wpool, \
             tc.tile_pool(name="obuf", bufs=4) as opool, \
             tc.tile_pool(name="ps", bufs=2, space="PSUM") as psp:

            wc_sb = cpool.tile([128, 2048], f32r)
            nc.gpsimd.dma_start(out=wc_sb[:], in_=wc_d.ap())
            b8_sb = cpool.tile([8, 128], f32r)
            nc.gpsimd.dma_start(out=b8_sb[:], in_=b8_d.ap())
            m8_sb = cpool.tile([8, 256], f32r)
            nc.gpsimd.dma_start(out=m8_sb[:], in_=m8_d.ap())
            stag_sb = cpool.tile([1, 1], f32)
            nc.sync.dma_start(out=stag_sb[:], in_=stag_d.ap()[0:1, 0:1])

            h_state = spool.tile([128, 2 * BS], f32r, tag="h")
            nc.gpsimd.dma_start(out=h_state[:], in_=hT0_d.ap())
            c_state = spool.tile([128, 2 * BS], f32, tag="c")
            nc.sync.dma_start(out=c_state[:], in_=cT0_d.ap())

            def emit_out(row_off):
                """Stage pre-update h_state (Pool) and DMA it feature-major.

                row_off is in units of outs_d rows (t * F).
                """
                stg = opool.tile([128, 2 * BS], f32, tag="stg")
                nc.gpsimd.tensor_copy(stg[:], h_state[:])
                if timing:
                    dst = outs_d.ap()[0:F]
                elif isinstance(row_off, int):
                    dst = outs_d.ap()[row_off:row_off + F]
                else:
                    dst = outs_d.ap()[ds(row_off, F)]
                nc.sync.dma_start(
                    out=dst.rearrange("(k p) b -> p k b", k=2),
                    in_=stg[:].rearrange("p (k b) -> p k b", k=2))

            def step(row_off):
                # start=True zeroes the WHOLE PSUM bank (bank-granular reset),
                # so only the first matmul of the step may carry it; all other
                # accumulations ride on the zeroed bank. k-major order lets
                # PE begin the 8 k0 matmuls as soon as h chunk0 is written.
                # Per-element accumulate order stays k0, k1, bias (= baseline).
                psT = psp.tile([128, 256], f32, tag="psT")
                for k in range(2):
                    for j in range(8):
                        nc.tensor.matmul(
                            psT[:, 32 * j:32 * (j + 1)],
                            lhsT=wc_sb[:, 1024 * k + 128 * j:1024 * k + 128 * (j + 1)],
                            rhs=h_state[:, BS * k:BS * (k + 1)],
                            start=(j == 0 and k == 0), stop=False,
                            skip_group_check=True)
                nc.tensor.matmul(psT[:], lhsT=b8_sb[:], rhs=m8_sb[:],
                                 start=False, stop=True, skip_group_check=True)
                # output of the PREVIOUS step (pre-update h_state): staged on
                # Pool + DMA'd; fully off the recurrence chain.
                in_bounds = not (isinstance(row_off, int)
                                 and row_off >= SEQ_LEN * F)
                if in_bounds:
                    emit_out(row_off)

                sg = wpool.tile([128, 192], f32, tag="sg")
                nc.scalar.activation(sg[:, 0:128], psT[:, 0:128], AF.Sigmoid)
                tg = wpool.tile([128, 64], f32, tag="tg")
                nc.scalar.activation(tg[:], psT[:, 192:256], AF.Tanh)
                nc.scalar.activation(sg[:, 128:192], psT[:, 128:192],
                                     AF.Sigmoid)
                t2 = wpool.tile([128, 64], f32, tag="t2")
                nc.gpsimd.tensor_mul(t2[:], sg[:, 64:128], c_state[:])
                t1 = wpool.tile([128, 64], f32, tag="t1")
                nc.vector.tensor_mul(t1[:], sg[:, 0:64], tg[:])
                nc.vector.tensor_add(c_state[:], t1[:], t2[:])
                tc2 = wpool.tile([128, 64], f32, tag="tc2")
                nc.scalar.activation(tc2[:], c_state[:], AF.Tanh)
                nc.vector.tensor_mul(h_state[:, 0:BS], sg[:, 128:160],
                                     tc2[:, 0:BS])
                nc.vector.tensor_mul(h_state[:, BS:2 * BS], sg[:, 160:192],
                                     tc2[:, BS:2 * BS])

            total = steps * reps
            n_loop = (total // UNROLL) * UNROLL
            if n_loop:
                with tc.For_i(0, n_loop * F, UNROLL * F) as i:
                    for u in range(UNROLL):
                        step(i + u * F)
            for u in range(total - n_loop):
                step((n_loop + u) * F)
            if total < SEQ_LEN:
                # row `steps` (= h_{steps+1}) not yet covered by a body
                emit_out(steps * F)

    nc.compile()
    return nc


def _get_nc(steps, reps=1, timing=False):
    key = (steps, reps, timing)
    if key not in _CACHE:
        _CACHE[key] = _build(steps, reps, timing)
    return _CACHE[key]


def _host_prep(x, last_feat, Wi, bi, W_ih, W_hh, b_ih, b_hh):
    x = np.asarray(x, np.float32)
    last_feat = np.asarray(last_feat, np.float32)
    Wi = np.asarray(Wi, np.float32); bi = np.asarray(bi, np.float32)
    W_ih = np.asarray(W_ih, np.float32); W_hh = np.asarray(W_hh, np.float32)
    b_ih = np.asarray(b_ih, np.float32); b_hh = np.asarray(b_hh, np.float32)

    z = x[0] @ Wi.T + bi                       # [B, F]
    init = np.where(z > 0, z, np.expm1(z)).astype(np.float32)  # elu

    bsum = b_ih + b_hh
    g0 = last_feat @ W_ih.T + init @ W_hh.T + bsum   # [B, 4F] order i,f,g,o
    i0, f0, g0g, o0 = (g0[:, 0:F], g0[:, F:2*F], g0[:, 2*F:3*F], g0[:, 3*F:4*F])
    c1 = _sigmoid(f0) * init + _sigmoid(i0) * np.tanh(g0g)
    h1 = (_sigmoid(o0) * np.tanh(c1)).astype(np.float32)
    c1 = c1.astype(np.float32)

    # combined recurrent weight, rows reordered [i, f, o, g]
    Wc = W_ih + W_hh                            # [4F, F]
    perm = np.concatenate([np.arange(0, F), np.arange(F, 2*F),
                           np.arange(3*F, 4*F), np.arange(2*F, 3*F)])
    wcT = np.ascontiguousarray(Wc[perm].T)      # [F, 4F] gate order i,f,o,g
    bias_perm = bsum[perm].astype(np.float32)   # [4F]
    return h1, c1, wcT, bias_perm


def _to_fmajor(a):
    """[BS, 256] batch-major -> [128, 2*BS] feature-chunk-major."""
    return np.ascontiguousarray(
        a.T.reshape(2, 128, BS).transpose(1, 0, 2).reshape(128, 2 * BS))


def kernel(x, last_feat, Wi, bi, W_ih, W_hh, b_ih, b_hh, Wo, bo,
           _steps=SEQ_LEN - 1, _reps=1, _timing=False):
    from concourse.bass_utils import run_bass_kernel_spmd

    h1, c1, wcT, bias_perm = _host_prep(x, last_feat, Wi, bi, W_ih, W_hh,
                                        b_ih, b_hh)
    # wc layout: wc[p, k*1024 + j*128 + q] = wcT[k*128 + p, j*128 + q]
    wc_host = np.ascontiguousarray(
        wcT.reshape(2, 128, 8, 128).transpose(1, 0, 2, 3).reshape(128, 2048))
    b8 = np.ascontiguousarray(bias_perm.reshape(8, 128))
    m8 = np.ascontiguousarray(
        np.kron(np.eye(8, dtype=np.float32), np.ones((1, BS), np.float32)))

    in_maps = []
    for ci in range(NCORES):
        s = slice(ci * BS, (ci + 1) * BS)
        in_maps.append(dict(
            hT0=_to_fmajor(h1[s]),
            cT0=_to_fmajor(c1[s]),
            wc=wc_host, b8=b8, m8=m8,
            stag=np.zeros((VERSION, _steps + _reps), np.float32)))

    nc = _get_nc(_steps, _reps, _timing)
    res = run_bass_kernel_spmd(nc, in_maps, core_ids=list(range(NCORES)))
    if _timing:
        return None

    # outs rows [t*F + f, b] = h_{t+1}[b, f]; device covers t = 0.._steps
    per_core = [r["outs"].reshape(SEQ_LEN, F, BS).transpose(0, 2, 1)
                for r in res.results]
    outs = np.concatenate(per_core, axis=1)  # [S, B, F]
    return np.ascontiguousarray(outs).reshape(B, SEQ_LEN, F)
